# revision 22
# baseline (speedup 1.0000x reference)
"""Trainium2 Bass kernel for nn_Encoder_Block (gnn_message_passing).

Graph-transformer encoder block:
  x1 = LN1(x); q,k,v = x1@W*; e = y@We
  attn = (q*k*scale) * (e+1)*e          (elementwise, per (nq,nk,c))
  edges = attn@Woe + boe
  p = softmax(attn, axis=nk); agg = sum_k p*v
  x2 = LN3(x1 + agg@Won + bon); x_out = LN5(x2 + MLP(x2))

Sharding: 8 cores; core i handles graph g=i//2, query rows [128*(i%2), 128*(i%2)+128).
Each core is fully independent (data parallel); host gathers.

Per-core layout: channels-on-partitions [c=128, nk] so that
  - softmax reduces along the free axis,
  - q-scaling is a per-partition scalar,
  - d4 = 4*(e^2+e) (.) kT serves directly as matmul lhsT for natural-layout edges.

Key identities (per query node nq):
  sq  = (2*(E+be) + 1)^2 = 4*G + 1 where G = e^2 + e     (one ACT op from PSUM)
  d4  = (sq - 1) (.) kT = 4*G (.) kT                     (one DVE scalar_tensor_tensor)
  attn[c,k] = q4[c] * d4[c,k],  q4 = (q+bq)*scale/4
  p = Exp(d4 * q4_scale), s = sum_k p                    (one ACT op w/ accum)
  a = sum_k p (.) vT                                     (one DVE tensor_tensor_reduce)
  edges[k,j] = sum_c d4[c,k] * (q4[c]*Woe[c,j]) + boe[j] (PE matmuls, natural layout out)
"""

import os

import numpy as np

# ---- problem constants (hardcoded per contract) ----
B, N, C, H = 4, 256, 128, 8
DK = C // H
SCALE = 1.0 / float(np.sqrt(DK))  # 0.25
S4 = SCALE / 4.0
HID = 4 * C
EPS = 1e-5
NCORES = 8
ROWS = N // 2  # 128 query rows per core
NBLK = 4       # query nodes per inner block

LAST_RESULTS = None  # test harness introspection


def _build_bass(repeat=1):
    import concourse.bacc as bacc
    import concourse.bass as bass
    import concourse.mybir as mybir
    import concourse.tile as tile
    from concourse.bass import ts
    from concourse.masks import make_identity

    fp32 = mybir.dt.float32
    bf16 = mybir.dt.bfloat16
    A = mybir.AluOpType
    F = mybir.ActivationFunctionType

    nc = bacc.Bacc("TRN2", target_bir_lowering=False, debug=False)

    def din(name, shape):
        return nc.dram_tensor(name, shape, fp32, kind="ExternalInput").ap()

    # y shard pre-transposed on host to (nq, c, nk) so the channel axis lands
    # on partitions with a contiguous cast-DMA (no on-device transpose).
    y_t = din("y_t", (ROWS, C, N))
    x_g = din("x_g", (N, C))
    x_q = din("x_q", (ROWS, C))
    Wq = din("Wq", (C, C)); bq = din("bq", (C,))
    Wk = din("Wk", (C, C)); bk = din("bk", (C,))
    Wv = din("Wv", (C, C)); bv = din("bv", (C,))
    We = din("We", (C, C)); be = din("be", (C,))
    Won = din("Won", (C, C)); bon = din("bon", (C,))
    Woe = din("Woe", (C, C)); boe = din("boe", (C,))
    W1 = din("W1", (C, HID)); b1 = din("b1", (HID,))
    W2 = din("W2", (HID, C)); b2 = din("b2", (C,))
    ln1_g = din("ln1_g", (C,)); ln1_b = din("ln1_b", (C,))
    ln3_g = din("ln3_g", (C,)); ln3_b = din("ln3_b", (C,))
    ln5_g = din("ln5_g", (C,)); ln5_b = din("ln5_b", (C,))

    edges_o = nc.dram_tensor("edges_o", (ROWS, N, C), fp32, kind="ExternalOutput").ap()
    xout_o = nc.dram_tensor("xout_o", (ROWS, C), fp32, kind="ExternalOutput").ap()

    def bcast_ap(vec_ap, p=128):
        # [C] dram vector broadcast across p partitions -> [p, C]
        return bass.AP(
            tensor=vec_ap.tensor, offset=vec_ap.offset, ap=[[0, p]] + list(vec_ap.ap)
        )

    def col_ap(vec_ap):
        # [C] dram vector as a [C, 1] partition-column
        return vec_ap.unsqueeze(1)

    with tile.TileContext(nc) as tc:
        with tc.tile_pool(name="singles", bufs=1) as singles:
            # ---------- load constants ----------
            We_bf = singles.tile([C, C], bf16)
            nc.gpsimd.dma_start(We_bf, We)
            Woe_bf = singles.tile([C, C], bf16)
            nc.gpsimd.dma_start(Woe_bf, Woe)
            Wq_t = singles.tile([C, C], fp32); nc.sync.dma_start(Wq_t, Wq)
            Wk_t = singles.tile([C, C], fp32); nc.sync.dma_start(Wk_t, Wk)
            Wv_t = singles.tile([C, C], fp32); nc.sync.dma_start(Wv_t, Wv)
            Won_t = singles.tile([C, C], fp32); nc.sync.dma_start(Won_t, Won)
            W1_t = singles.tile([C, HID], fp32); nc.sync.dma_start(W1_t, W1)
            W2_t = singles.tile([C, HID // C, C], fp32)
            nc.sync.dma_start(W2_t, W2.rearrange("(t p) c -> p t c", p=C))

            bq_c = singles.tile([C, 1], fp32); nc.sync.dma_start(bq_c, col_ap(bq))
            bk_c = singles.tile([C, 1], fp32); nc.sync.dma_start(bk_c, col_ap(bk))
            bv_c = singles.tile([C, 1], fp32); nc.sync.dma_start(bv_c, col_ap(bv))
            be_c = singles.tile([C, 1], fp32); nc.sync.dma_start(be_c, col_ap(be))
            bon_c = singles.tile([C, 1], fp32); nc.sync.dma_start(bon_c, col_ap(bon))
            b2_c = singles.tile([C, 1], fp32); nc.sync.dma_start(b2_c, col_ap(b2))
            b1_t = singles.tile([C, HID // C], fp32)
            nc.sync.dma_start(b1_t, b1.rearrange("(t p) -> p t", p=C))

            # boe repeated 4x in partition 0 (rhs of the rank-1 bias matmul); bf16
            boe4 = singles.tile([1, 4 * C], bf16)
            for r in range(4):
                nc.gpsimd.dma_start(boe4[:, r * C : (r + 1) * C], boe.unsqueeze(0))
            ones1 = singles.tile([1, C], bf16)
            nc.vector.memset(ones1, 1.0)

            g1_b = singles.tile([C, C], fp32); nc.sync.dma_start(g1_b, bcast_ap(ln1_g))
            b1_b = singles.tile([C, C], fp32); nc.sync.dma_start(b1_b, bcast_ap(ln1_b))
            g3_b = singles.tile([C, C], fp32); nc.sync.dma_start(g3_b, bcast_ap(ln3_g))
            b3_b = singles.tile([C, C], fp32); nc.sync.dma_start(b3_b, bcast_ap(ln3_b))
            g5_b = singles.tile([C, C], fp32); nc.sync.dma_start(g5_b, bcast_ap(ln5_g))
            b5_b = singles.tile([C, C], fp32); nc.sync.dma_start(b5_b, bcast_ap(ln5_b))

            eps_t = singles.tile([C, 1], fp32)
            nc.vector.memset(eps_t, EPS)

            identity = singles.tile([C, C], fp32)
            make_identity(nc, identity[:])

            # b2e = 2*be + 1  (bias for the Square trick)
            b2e = singles.tile([C, 1], fp32)
            nc.vector.tensor_scalar(b2e, be_c, 2.0, 1.0, A.mult, A.add)
            # bq4 = bq * scale/4
            bq4 = singles.tile([C, 1], fp32)
            nc.vector.tensor_scalar_mul(bq4, bq_c, S4)

            # persistent activations
            kT_bf = singles.tile([C, N], bf16)
            vT_bf = singles.tile([C, N], bf16)
            q4 = singles.tile([C, ROWS], fp32)
            aggT = singles.tile([C, ROWS], fp32)
            x1q = singles.tile([ROWS, C], fp32)

            def ln_tile(pool, out, x_tile, g_bc, b_bc):
                st = pool.tile([C, 6], fp32, tag="ln_stats")
                nc.vector.bn_stats(st, x_tile)
                mv = pool.tile([C, 2], fp32, tag="ln_mv")
                nc.vector.bn_aggr(mv, st)
                sd = pool.tile([C, 1], fp32, tag="ln_sd")
                nc.scalar.activation(sd, mv[:, 1:2], F.Sqrt, bias=eps_t, scale=1.0)
                rstd = pool.tile([C, 1], fp32, tag="ln_rstd")
                nc.vector.reciprocal(rstd, sd)
                nc.vector.tensor_scalar(
                    out, x_tile, mv[:, 0:1], rstd, A.subtract, A.mult
                )
                nc.vector.tensor_mul(out, out, g_bc)
                nc.vector.tensor_add(out, out, b_bc)

            # ---------- setup: x-path projections ----------
            with (
                tc.tile_pool(name="setup_sb", bufs=2) as ssb,
                tc.tile_pool(name="setup_ps", bufs=2, space="PSUM") as sps,
            ):
                # LN1 over the full graph (for k, v) and over this core's rows (q)
                x1T = singles.tile([C, N], fp32)
                for t in range(2):
                    xg_t = ssb.tile([C, C], fp32, tag="xg")
                    nc.sync.dma_start(xg_t, x_g[ts(t, C)])
                    x1g = ssb.tile([C, C], fp32, tag="x1g")
                    ln_tile(ssb, x1g, xg_t, g1_b, b1_b)
                    ps = sps.tile([C, C], fp32, tag="tps")
                    nc.tensor.transpose(ps, x1g, identity)
                    nc.scalar.copy(x1T[:, ts(t, C)], ps)

                xq_t = ssb.tile([C, C], fp32, tag="xq")
                nc.sync.dma_start(xq_t, x_q)
                ln_tile(ssb, x1q, xq_t, g1_b, b1_b)
                x1qT = singles.tile([C, ROWS], fp32)
                ps = sps.tile([C, C], fp32, tag="tps")
                nc.tensor.transpose(ps, x1q, identity)
                nc.scalar.copy(x1qT, ps)

                # kT/vT (full graph), q4 (this core's rows)
                kv_ps = sps.tile([C, N], fp32, tag="kv")
                nc.tensor.matmul(kv_ps, Wk_t, x1T, start=True, stop=True)
                nc.scalar.activation(kT_bf, kv_ps, F.Identity, bias=bk_c, scale=1.0)
                kv_ps2 = sps.tile([C, N], fp32, tag="kv")
                nc.tensor.matmul(kv_ps2, Wv_t, x1T, start=True, stop=True)
                nc.scalar.activation(vT_bf, kv_ps2, F.Identity, bias=bv_c, scale=1.0)
                q_ps = sps.tile([C, ROWS], fp32, tag="q")
                nc.tensor.matmul(q_ps, Wq_t, x1qT, start=True, stop=True)
                nc.scalar.activation(q4, q_ps, F.Identity, bias=bq4, scale=S4)

            # ---------- main loop over query nodes ----------
            with (
                tc.tile_pool(name="yt", bufs=3) as yt_pool,
                tc.tile_pool(name="sqp", bufs=2) as sq_pool,
                tc.tile_pool(name="d4p", bufs=2 * NBLK) as d4_pool,
                tc.tile_pool(name="dqwp", bufs=2 * NBLK) as dqw_pool,
                tc.tile_pool(name="pp", bufs=2 * NBLK) as p_pool,
                tc.tile_pool(name="pvp", bufs=2) as pv_pool,
                tc.tile_pool(name="edsb", bufs=3) as edsb_pool,
                tc.tile_pool(name="stat", bufs=3) as stat_pool,
                tc.tile_pool(name="psE", bufs=2, space="PSUM") as psum_e,
                tc.tile_pool(name="psED", bufs=2, space="PSUM") as psum_ed,
            ):
                for blk in range(repeat * (ROWS // NBLK)):
                    nq0 = (blk % (ROWS // NBLK)) * NBLK
                    yT = yt_pool.tile([C, NBLK, N], bf16)
                    nc.gpsimd.dma_start(
                        yT, y_t[nq0 : nq0 + NBLK].rearrange("b c k -> c b k")
                    )

                    e_ps = psum_e.tile([C, NBLK, N], fp32)
                    for half in range(NBLK // 2):
                        nc.tensor.matmul(
                            e_ps[:, 2 * half : 2 * half + 2],
                            We_bf,
                            yT[:, 2 * half : 2 * half + 2],
                            start=True,
                            stop=True,
                        )
                    sq = sq_pool.tile([C, NBLK, N], bf16)
                    nc.scalar.activation(sq, e_ps, F.Square, bias=b2e, scale=2.0)

                    ed_ps = psum_ed.tile([C, NBLK, 2, C], fp32)
                    s_blk = stat_pool.tile([C, NBLK], fp32, tag="s")
                    a_blk = stat_pool.tile([C, NBLK], fp32, tag="a")
                    for bi in range(NBLK):
                        nq = nq0 + bi
                        if bi % 2 == 0:
                            # bias prefill: one rank-1 matmul per pair of nodes
                            nc.tensor.matmul(
                                ed_ps[:, bi : bi + 2],
                                ones1,
                                boe4,
                                start=True,
                                stop=False,
                            )
                        d4 = d4_pool.tile([C, N], bf16)
                        nc.vector.scalar_tensor_tensor(
                            d4, sq[:, bi], -1.0, kT_bf, A.add, A.mult
                        )
                        dqw = dqw_pool.tile([C, C], bf16)
                        nc.vector.tensor_scalar_mul(dqw, Woe_bf, q4[:, nq : nq + 1])
                        for t in range(2):
                            nc.tensor.matmul(
                                ed_ps[:, bi, t],
                                d4[:, ts(t, C)],
                                dqw,
                                start=False,
                                stop=(bi % 2 == 1 and t == 1),
                            )
                        p_t = p_pool.tile([C, N], bf16)
                        nc.scalar.activation(
                            p_t,
                            d4,
                            F.Exp,
                            scale=q4[:, nq : nq + 1],
                            accum_out=s_blk[:, bi : bi + 1],
                        )
                        pv = pv_pool.tile([C, N], bf16)
                        nc.vector.scalar_tensor_tensor(
                            pv,
                            p_t,
                            1.0,
                            vT_bf,
                            A.mult,
                            A.mult,
                            accum_out=a_blk[:, bi : bi + 1],
                        )

                    ed_sb = edsb_pool.tile([C, NBLK, 2, C], bf16)
                    if blk % 2 == 0:
                        nc.scalar.copy(ed_sb, ed_ps)
                    else:
                        nc.vector.tensor_copy(ed_sb, ed_ps)
                    nc.gpsimd.dma_start(
                        edges_o[nq0 : nq0 + NBLK].rearrange(
                            "b (t p) c -> p b t c", p=C
                        ),
                        ed_sb,
                    )

                    rec = stat_pool.tile([C, NBLK], fp32, tag="rec")
                    nc.vector.reciprocal(rec, s_blk)
                    nc.vector.tensor_mul(aggT[:, nq0 : nq0 + NBLK], a_blk, rec)

            # ---------- tail: node update + MLP ----------
            with (
                tc.tile_pool(name="tail_sb", bufs=1) as tsb,
                tc.tile_pool(name="tail_ps", bufs=1, space="PSUM") as tps,
            ):
                nodes_ps = tps.tile([C, ROWS], fp32, tag="m")
                nc.tensor.matmul(nodes_ps, Won_t, aggT, start=True, stop=True)
                nodesT = tsb.tile([C, ROWS], fp32, tag="t0")
                nc.scalar.activation(nodesT, nodes_ps, F.Identity, bias=bon_c, scale=1.0)
                ps = tps.tile([ROWS, C], fp32, tag="tr")
                nc.tensor.transpose(ps, nodesT, identity)
                x2 = tsb.tile([ROWS, C], fp32, tag="x2")
                nc.vector.tensor_add(x2, x1q, ps)
                x2n = tsb.tile([ROWS, C], fp32, tag="x2n")
                ln_tile(tsb, x2n, x2, g3_b, b3_b)
                ps2 = tps.tile([ROWS, C], fp32, tag="tr")
                nc.tensor.transpose(ps2, x2n, identity)
                x2nT = tsb.tile([C, ROWS], fp32, tag="t1")
                nc.scalar.copy(x2nT, ps2)

                h_ps = tps.tile([C, HID // C, ROWS], fp32, tag="h")
                hr = tsb.tile([C, HID // C, ROWS], fp32, tag="hr")
                for m in range(HID // C):
                    nc.tensor.matmul(
                        h_ps[:, m], W1_t[:, ts(m, C)], x2nT, start=True, stop=True
                    )
                    nc.scalar.activation(
                        hr[:, m], h_ps[:, m], F.Relu, bias=b1_t[:, m : m + 1], scale=1.0
                    )
                mlp_ps = tps.tile([C, ROWS], fp32, tag="m")
                for t in range(HID // C):
                    nc.tensor.matmul(
                        mlp_ps,
                        W2_t[:, t],
                        hr[:, t],
                        start=(t == 0),
                        stop=(t == HID // C - 1),
                    )
                mlpT = tsb.tile([C, ROWS], fp32, tag="t0")
                nc.scalar.activation(mlpT, mlp_ps, F.Identity, bias=b2_c, scale=1.0)
                ps3 = tps.tile([ROWS, C], fp32, tag="tr")
                nc.tensor.transpose(ps3, mlpT, identity)
                x3 = tsb.tile([ROWS, C], fp32, tag="x3")
                nc.vector.tensor_add(x3, x2n, ps3)
                xout_t = tsb.tile([ROWS, C], fp32, tag="xo")
                ln_tile(tsb, xout_t, x3, g5_b, b5_b)
                nc.sync.dma_start(xout_o, xout_t)

    nc.compile()
    return nc


_NC_CACHE = {}


def _get_nc(repeat=1):
    if repeat not in _NC_CACHE:
        _NC_CACHE[repeat] = _build_bass(repeat)
    return _NC_CACHE[repeat]


def _shard_inputs(inputs):
    """Build per-core in_maps from full inputs."""
    x = np.asarray(inputs["x"], np.float32)
    y = np.asarray(inputs["y"], np.float32)
    weights = {
        k: np.asarray(inputs[k], np.float32)
        for k in (
            "Wq", "bq", "Wk", "bk", "Wv", "bv", "We", "be", "Won", "bon",
            "Woe", "boe", "W1", "b1", "W2", "b2",
            "ln1_g", "ln1_b", "ln3_g", "ln3_b", "ln5_g", "ln5_b",
        )
    }
    in_maps = []
    for core in range(NCORES):
        g, hhalf = core // 2, core % 2
        r0 = hhalf * ROWS
        m = {
            # (nq, c, nk): channel axis onto partitions, contiguous key axis
            "y_t": np.ascontiguousarray(
                y[g, r0 : r0 + ROWS].transpose(0, 2, 1)
            ),
            "x_g": np.ascontiguousarray(x[g]),
            "x_q": np.ascontiguousarray(x[g, r0 : r0 + ROWS]),
        }
        m.update(weights)
        in_maps.append(m)
    return in_maps


def benchmark(inputs, iters=8, repeat=1):
    """Best wall time of one sharded dispatch of the repeat-variant NEFF
    (includes ~fixed host/tunnel overhead; difference two repeat values to
    isolate on-device execution time)."""
    import time

    import jax
    import numpy as np
    from jax.sharding import Mesh, PartitionSpec
    from jax.experimental.shard_map import shard_map

    import concourse.mybir as mybir
    from concourse.bass2jax import (
        _bass_exec_p,
        install_neuronx_cc_hook,
        partition_id_tensor,
    )

    install_neuronx_cc_hook()
    nc = _get_nc(repeat)
    in_maps = _shard_inputs(inputs)
    partition_name = nc.partition_id_tensor.name if nc.partition_id_tensor else None

    in_names, out_names, out_avals, zero_outs = [], [], [], []
    for alloc in nc.m.functions[0].allocations:
        if not isinstance(alloc, mybir.MemoryLocationSet):
            continue
        name = alloc.memorylocations[0].name
        if alloc.kind == "ExternalInput":
            if name != partition_name:
                in_names.append(name)
        elif alloc.kind == "ExternalOutput":
            out_names.append(name)
            shape = tuple(alloc.tensor_shape)
            dtype = mybir.dt.np(alloc.dtype)
            out_avals.append(jax.core.ShapedArray(shape, dtype))
            zero_outs.append(np.zeros(shape, dtype))
    n_params = len(in_names)
    all_in_names = in_names + out_names
    if partition_name is not None:
        all_in_names = all_in_names + [partition_name]

    def _body(*args):
        operands = list(args)
        if partition_name is not None:
            operands.append(partition_id_tensor())
        outs = _bass_exec_p.bind(
            *operands,
            out_avals=tuple(out_avals),
            in_names=tuple(all_in_names),
            out_names=tuple(out_names),
            lowering_input_output_aliases=(),
            sim_require_finite=True,
            sim_require_nnan=True,
            nc=nc,
        )
        return tuple(outs)

    devices = jax.devices()[:NCORES]
    mesh = Mesh(np.asarray(devices), ("core",))
    spec = PartitionSpec("core")
    n_all = n_params + len(out_names)

    concat_in = [
        np.concatenate([np.asarray(in_maps[c][nm]) for c in range(NCORES)], axis=0)
        for nm in in_names
    ]
    concat_zero = [
        np.zeros((NCORES * z.shape[0], *z.shape[1:]), z.dtype) for z in zero_outs
    ]
    sharding = jax.sharding.NamedSharding(mesh, spec)
    dev_args = [jax.device_put(a, sharding) for a in concat_in + concat_zero]

    fn = jax.jit(
        shard_map(
            _body, mesh=mesh, in_specs=(spec,) * n_all,
            out_specs=(spec,) * len(out_names), check_rep=False,
        )
    )
    fn(*dev_args)[0].block_until_ready()  # compile+warm
    best = float("inf")
    for _ in range(iters):
        t0 = time.perf_counter()
        fn(*dev_args)[0].block_until_ready()
        best = min(best, time.perf_counter() - t0)
    return best * 1e9





def kernel(**inputs):
    global LAST_RESULTS
    from concourse import bass_utils

    nc = _get_nc()
    in_maps = _shard_inputs(inputs)
    trace = os.environ.get("KERNEL_TRACE", "0") == "1"
    res = bass_utils.run_bass_kernel_spmd(
        nc, in_maps, core_ids=list(range(NCORES)), trace=trace
    )
    LAST_RESULTS = res

    x_out = np.empty((B, N, C), np.float32)
    edges = np.empty((B, N, N, C), np.float32)
    for core in range(NCORES):
        g, hhalf = core // 2, core % 2
        r0 = hhalf * ROWS
        x_out[g, r0 : r0 + ROWS] = res.results[core]["xout_o"]
        edges[g, r0 : r0 + ROWS] = res.results[core]["edges_o"]
    return (x_out, edges)


# revision 32
# speedup vs baseline: 126.9429x; 126.9429x over previous
"""Trainium2 Bass kernel for nn_Encoder_Block (gnn_message_passing).

Graph-transformer encoder block:
  x1 = LN1(x); q,k,v = x1@W*; e = y@We
  attn = (q*k*scale) * (e+1)*e          (elementwise, per (nq,nk,c))
  edges = attn@Woe + boe
  p = softmax(attn, axis=nk); agg = sum_k p*v
  x2 = LN3(x1 + agg@Won + bon); x_out = LN5(x2 + MLP(x2))

Sharding: 8 cores; core i handles graph g=i//2, query rows [128*(i%2), 128*(i%2)+128).
Each core is fully independent (data parallel); host gathers.

Per-core layout: channels-on-partitions [c=128, nk] so that
  - softmax reduces along the free axis,
  - q-scaling is a per-partition scalar,
  - d4 = 4*(e^2+e) (.) kT serves directly as matmul lhsT for natural-layout edges.

Key identities (per query node nq):
  sq  = (2*(E+be) + 1)^2 = 4*G + 1 where G = e^2 + e     (one ACT op from PSUM)
  d4  = (sq - 1) (.) kT = 4*G (.) kT                     (one DVE scalar_tensor_tensor)
  attn[c,k] = q4[c] * d4[c,k],  q4 = (q+bq)*scale/4
  p = Exp(d4 * q4_scale), s = sum_k p                    (one ACT op w/ accum)
  a = sum_k p (.) vT                                     (one DVE tensor_tensor_reduce)
  edges[k,j] = sum_c d4[c,k] * (q4[c]*Woe[c,j]) + boe[j] (PE matmuls, natural layout out)
"""

import os

import numpy as np

# ---- problem constants (hardcoded per contract) ----
B, N, C, H = 4, 256, 128, 8
DK = C // H
SCALE = 1.0 / float(np.sqrt(DK))  # 0.25
S4 = SCALE / 4.0
HID = 4 * C
EPS = 1e-5
NCORES = 8
ROWS = N // 2  # 128 query rows per core
NBLK = 4       # query nodes per inner block

LAST_RESULTS = None  # test harness introspection


def _build_bass(repeat=1):
    hwdge = os.environ.get("KERNEL_HWDGE", "0") == "1"
    perf_probe = os.environ.get("KERNEL_PERF_PROBE", "")
    import concourse.bacc as bacc
    import concourse.bass as bass
    import concourse.mybir as mybir
    import concourse.tile as tile
    from concourse.bass import ts
    from concourse.masks import make_identity

    fp32 = mybir.dt.float32
    bf16 = mybir.dt.bfloat16
    A = mybir.AluOpType
    F = mybir.ActivationFunctionType

    nc = bacc.Bacc("TRN2", target_bir_lowering=False, debug=False)

    def din(name, shape):
        return nc.dram_tensor(name, shape, fp32, kind="ExternalInput").ap()

    # y shard pre-transposed on host to (nq, c, nk) so the channel axis lands
    # on partitions with a contiguous cast-DMA (no on-device transpose).
    y_t = din("y_t", (ROWS, C, N))
    x_g = din("x_g", (N, C))
    x_q = din("x_q", (ROWS, C))
    Wq = din("Wq", (C, C)); bq = din("bq", (C,))
    Wk = din("Wk", (C, C)); bk = din("bk", (C,))
    Wv = din("Wv", (C, C)); bv = din("bv", (C,))
    We = din("We", (C, C)); be = din("be", (C,))
    Won = din("Won", (C, C)); bon = din("bon", (C,))
    Woe = din("Woe", (C, C)); boe = din("boe", (C,))
    W1 = din("W1", (C, HID)); b1 = din("b1", (HID,))
    W2 = din("W2", (HID, C)); b2 = din("b2", (C,))
    ln1_g = din("ln1_g", (C,)); ln1_b = din("ln1_b", (C,))
    ln3_g = din("ln3_g", (C,)); ln3_b = din("ln3_b", (C,))
    ln5_g = din("ln5_g", (C,)); ln5_b = din("ln5_b", (C,))

    edges_o = nc.dram_tensor("edges_o", (ROWS, N, C), fp32, kind="ExternalOutput").ap()
    xout_o = nc.dram_tensor("xout_o", (ROWS, C), fp32, kind="ExternalOutput").ap()

    def bcast_ap(vec_ap, p=128):
        # [C] dram vector broadcast across p partitions -> [p, C]
        return bass.AP(
            tensor=vec_ap.tensor, offset=vec_ap.offset, ap=[[0, p]] + list(vec_ap.ap)
        )

    def col_ap(vec_ap):
        # [C] dram vector as a [C, 1] partition-column
        return vec_ap.unsqueeze(1)

    with tile.TileContext(nc) as tc:
        with tc.tile_pool(name="singles", bufs=1) as singles:
            # ---------- load constants ----------
            We_bf = singles.tile([C, C], bf16)
            nc.gpsimd.dma_start(We_bf, We)
            We_f = singles.tile([C, C], fp32)
            nc.sync.dma_start(We_f, We)
            Woe_bf = singles.tile([C, C], bf16)
            nc.gpsimd.dma_start(Woe_bf, Woe)
            Wq_t = singles.tile([C, C], fp32); nc.sync.dma_start(Wq_t, Wq)
            Wk_t = singles.tile([C, C], fp32); nc.sync.dma_start(Wk_t, Wk)
            Wv_t = singles.tile([C, C], fp32); nc.sync.dma_start(Wv_t, Wv)
            Won_t = singles.tile([C, C], fp32); nc.sync.dma_start(Won_t, Won)
            W1_t = singles.tile([C, HID], fp32); nc.sync.dma_start(W1_t, W1)
            W2_t = singles.tile([C, HID // C, C], fp32)
            nc.sync.dma_start(W2_t, W2.rearrange("(t p) c -> p t c", p=C))

            bq_c = singles.tile([C, 1], fp32); nc.sync.dma_start(bq_c, col_ap(bq))
            bk_c = singles.tile([C, 1], fp32); nc.sync.dma_start(bk_c, col_ap(bk))
            bv_c = singles.tile([C, 1], fp32); nc.sync.dma_start(bv_c, col_ap(bv))
            be_c = singles.tile([C, 1], fp32); nc.sync.dma_start(be_c, col_ap(be))
            bon_c = singles.tile([C, 1], fp32); nc.sync.dma_start(bon_c, col_ap(bon))
            b2_c = singles.tile([C, 1], fp32); nc.sync.dma_start(b2_c, col_ap(b2))
            b1_t = singles.tile([C, HID // C], fp32)
            nc.sync.dma_start(b1_t, b1.rearrange("(t p) -> p t", p=C))

            # boe repeated 4x in partition 0 (rhs of the rank-1 bias matmul); bf16
            boe4 = singles.tile([1, 4 * C], bf16)
            for r in range(4):
                nc.gpsimd.dma_start(boe4[:, r * C : (r + 1) * C], boe.unsqueeze(0))
            ones1 = singles.tile([1, C], bf16)
            nc.vector.memset(ones1, 1.0)

            g1_b = singles.tile([C, C], fp32); nc.sync.dma_start(g1_b, bcast_ap(ln1_g))
            b1_b = singles.tile([C, C], fp32); nc.sync.dma_start(b1_b, bcast_ap(ln1_b))
            g3_b = singles.tile([C, C], fp32); nc.sync.dma_start(g3_b, bcast_ap(ln3_g))
            b3_b = singles.tile([C, C], fp32); nc.sync.dma_start(b3_b, bcast_ap(ln3_b))
            g5_b = singles.tile([C, C], fp32); nc.sync.dma_start(g5_b, bcast_ap(ln5_g))
            b5_b = singles.tile([C, C], fp32); nc.sync.dma_start(b5_b, bcast_ap(ln5_b))

            eps_t = singles.tile([C, 1], fp32)
            nc.vector.memset(eps_t, EPS)

            identity = singles.tile([C, C], fp32)
            make_identity(nc, identity[:])

            # b2e = 2*be + 1  (bias for the Square trick)
            b2e = singles.tile([C, 1], fp32)
            nc.vector.tensor_scalar(b2e, be_c, 2.0, 1.0, A.mult, A.add)
            # bq4 = bq * scale/4
            bq4 = singles.tile([C, 1], fp32)
            nc.vector.tensor_scalar_mul(bq4, bq_c, S4)

            # persistent activations
            kT_bf = singles.tile([C, N], bf16)
            vT_bf = singles.tile([C, N], bf16)
            q4 = singles.tile([C, ROWS], fp32)
            aggT = singles.tile([C, ROWS], fp32)
            if perf_probe == "skip_softmax":
                nc.vector.memset(aggT, 0.0)
            x1q = singles.tile([ROWS, C], fp32)

            def ln_tile(pool, out, x_tile, g_bc, b_bc):
                st = pool.tile([C, 6], fp32, tag="ln_stats")
                nc.vector.bn_stats(st, x_tile)
                mv = pool.tile([C, 2], fp32, tag="ln_mv")
                nc.vector.bn_aggr(mv, st)
                sd = pool.tile([C, 1], fp32, tag="ln_sd")
                nc.scalar.activation(sd, mv[:, 1:2], F.Sqrt, bias=eps_t, scale=1.0)
                rstd = pool.tile([C, 1], fp32, tag="ln_rstd")
                nc.vector.reciprocal(rstd, sd)
                nc.vector.tensor_scalar(
                    out, x_tile, mv[:, 0:1], rstd, A.subtract, A.mult
                )
                nc.vector.tensor_mul(out, out, g_bc)
                nc.vector.tensor_add(out, out, b_bc)

            # ---------- setup: x-path projections ----------
            with (
                tc.tile_pool(name="setup_sb", bufs=2) as ssb,
                tc.tile_pool(name="setup_ps", bufs=2, space="PSUM") as sps,
            ):
                # LN1 over the full graph (for k, v) and over this core's rows (q)
                x1T = singles.tile([C, N], fp32)
                for t in range(2):
                    xg_t = ssb.tile([C, C], fp32, tag="xg")
                    nc.sync.dma_start(xg_t, x_g[ts(t, C)])
                    x1g = ssb.tile([C, C], fp32, tag="x1g")
                    ln_tile(ssb, x1g, xg_t, g1_b, b1_b)
                    ps = sps.tile([C, C], fp32, tag="tps")
                    nc.tensor.transpose(ps, x1g, identity)
                    nc.scalar.copy(x1T[:, ts(t, C)], ps)

                xq_t = ssb.tile([C, C], fp32, tag="xq")
                nc.sync.dma_start(xq_t, x_q)
                ln_tile(ssb, x1q, xq_t, g1_b, b1_b)
                x1qT = singles.tile([C, ROWS], fp32)
                ps = sps.tile([C, C], fp32, tag="tps")
                nc.tensor.transpose(ps, x1q, identity)
                nc.scalar.copy(x1qT, ps)

                # kT/vT (full graph), q4 (this core's rows)
                kv_ps = sps.tile([C, N], fp32, tag="kv")
                nc.tensor.matmul(kv_ps, Wk_t, x1T, start=True, stop=True)
                nc.scalar.activation(kT_bf, kv_ps, F.Identity, bias=bk_c, scale=1.0)
                kv_ps2 = sps.tile([C, N], fp32, tag="kv")
                nc.tensor.matmul(kv_ps2, Wv_t, x1T, start=True, stop=True)
                nc.scalar.activation(vT_bf, kv_ps2, F.Identity, bias=bv_c, scale=1.0)
                q_ps = sps.tile([C, ROWS], fp32, tag="q")
                nc.tensor.matmul(q_ps, Wq_t, x1qT, start=True, stop=True)
                nc.scalar.activation(q4, q_ps, F.Identity, bias=bq4, scale=S4)

            # ---------- main loop over query nodes ----------
            nbuf = int(os.environ.get("KERNEL_BUFS", "3"))
            with (
                tc.tile_pool(name="yt", bufs=nbuf) as yt_pool,
                tc.tile_pool(name="sqp", bufs=nbuf) as sq_pool,
                tc.tile_pool(name="d4p", bufs=nbuf * NBLK) as d4_pool,
                tc.tile_pool(name="dqwp", bufs=nbuf * NBLK) as dqw_pool,
                tc.tile_pool(name="pp", bufs=nbuf * NBLK) as p_pool,
                tc.tile_pool(name="pvp", bufs=3) as pv_pool,
                tc.tile_pool(name="edsb", bufs=nbuf) as edsb_pool,
                tc.tile_pool(name="stat", bufs=4) as stat_pool,
                tc.tile_pool(name="psE", bufs=2, space="PSUM") as psum_e,
                tc.tile_pool(name="psED", bufs=2, space="PSUM") as psum_ed,
            ):
                for blk in range(repeat * (ROWS // NBLK)):
                    nq0 = (blk % (ROWS // NBLK)) * NBLK
                    yT = yt_pool.tile([C, NBLK, N], fp32 if hwdge else bf16)
                    dma_in = nc.sync.dma_start if hwdge else nc.gpsimd.dma_start
                    dma_in(yT, y_t[nq0 : nq0 + NBLK].rearrange("b c k -> c b k"))

                    e_ps = psum_e.tile([C, NBLK, N], fp32)
                    for half in range(NBLK // 2):
                        nc.tensor.matmul(
                            e_ps[:, 2 * half : 2 * half + 2],
                            We_f if hwdge else We_bf,
                            yT[:, 2 * half : 2 * half + 2],
                            start=True,
                            stop=True,
                        )
                    sq = sq_pool.tile([C, NBLK, N], bf16)
                    nc.scalar.activation(sq, e_ps, F.Square, bias=b2e, scale=2.0)

                    ed_ps = psum_ed.tile([C, NBLK, 2, C], fp32)
                    s_blk = stat_pool.tile([C, NBLK], fp32, tag="s")
                    a_blk = stat_pool.tile([C, NBLK], fp32, tag="a")
                    for bi in range(NBLK):
                        nq = nq0 + bi
                        if bi % 2 == 0:
                            # bias prefill: one rank-1 matmul per pair of nodes
                            nc.tensor.matmul(
                                ed_ps[:, bi : bi + 2],
                                ones1,
                                boe4,
                                start=True,
                                stop=False,
                            )
                        d4 = d4_pool.tile([C, N], bf16)
                        nc.vector.scalar_tensor_tensor(
                            d4, sq[:, bi], -1.0, kT_bf, A.add, A.mult
                        )
                        dqw = dqw_pool.tile([C, C], bf16)
                        nc.vector.tensor_scalar_mul(dqw, Woe_bf, q4[:, nq : nq + 1])
                        for t in range(2):
                            nc.tensor.matmul(
                                ed_ps[:, bi, t],
                                d4[:, ts(t, C)],
                                dqw,
                                start=False,
                                stop=(bi % 2 == 1 and t == 1),
                            )
                        if perf_probe != "skip_softmax":
                            p_t = p_pool.tile([C, N], bf16)
                            nc.scalar.activation(
                                p_t,
                                d4,
                                F.Exp,
                                scale=q4[:, nq : nq + 1],
                                accum_out=s_blk[:, bi : bi + 1],
                            )
                            pv = pv_pool.tile([C, N], bf16)
                            nc.vector.scalar_tensor_tensor(
                                pv,
                                p_t,
                                1.0,
                                vT_bf,
                                A.mult,
                                A.mult,
                                accum_out=a_blk[:, bi : bi + 1],
                            )

                    ed_sb = edsb_pool.tile([C, NBLK, 2, C], fp32 if hwdge else bf16)
                    if blk % 2 == 0:
                        nc.scalar.copy(ed_sb, ed_ps)
                    else:
                        nc.vector.tensor_copy(ed_sb, ed_ps)
                    dma_out = nc.sync.dma_start if hwdge else nc.gpsimd.dma_start
                    dma_out(
                        edges_o[nq0 : nq0 + NBLK].rearrange(
                            "b (t p) c -> p b t c", p=C
                        ),
                        ed_sb,
                    )

                    if perf_probe != "skip_softmax":
                        rec = stat_pool.tile([C, NBLK], fp32, tag="rec")
                        nc.vector.reciprocal(rec, s_blk)
                        nc.vector.tensor_mul(aggT[:, nq0 : nq0 + NBLK], a_blk, rec)

            # ---------- tail: node update + MLP ----------
            with (
                tc.tile_pool(name="tail_sb", bufs=1) as tsb,
                tc.tile_pool(name="tail_ps", bufs=1, space="PSUM") as tps,
            ):
                nodes_ps = tps.tile([C, ROWS], fp32, tag="m")
                nc.tensor.matmul(nodes_ps, Won_t, aggT, start=True, stop=True)
                nodesT = tsb.tile([C, ROWS], fp32, tag="t0")
                nc.scalar.activation(nodesT, nodes_ps, F.Identity, bias=bon_c, scale=1.0)
                ps = tps.tile([ROWS, C], fp32, tag="tr")
                nc.tensor.transpose(ps, nodesT, identity)
                x2 = tsb.tile([ROWS, C], fp32, tag="x2")
                nc.vector.tensor_add(x2, x1q, ps)
                x2n = tsb.tile([ROWS, C], fp32, tag="x2n")
                ln_tile(tsb, x2n, x2, g3_b, b3_b)
                ps2 = tps.tile([ROWS, C], fp32, tag="tr")
                nc.tensor.transpose(ps2, x2n, identity)
                x2nT = tsb.tile([C, ROWS], fp32, tag="t1")
                nc.scalar.copy(x2nT, ps2)

                h_ps = tps.tile([C, HID // C, ROWS], fp32, tag="h")
                hr = tsb.tile([C, HID // C, ROWS], fp32, tag="hr")
                for m in range(HID // C):
                    nc.tensor.matmul(
                        h_ps[:, m], W1_t[:, ts(m, C)], x2nT, start=True, stop=True
                    )
                    nc.scalar.activation(
                        hr[:, m], h_ps[:, m], F.Relu, bias=b1_t[:, m : m + 1], scale=1.0
                    )
                mlp_ps = tps.tile([C, ROWS], fp32, tag="m")
                for t in range(HID // C):
                    nc.tensor.matmul(
                        mlp_ps,
                        W2_t[:, t],
                        hr[:, t],
                        start=(t == 0),
                        stop=(t == HID // C - 1),
                    )
                mlpT = tsb.tile([C, ROWS], fp32, tag="t0")
                nc.scalar.activation(mlpT, mlp_ps, F.Identity, bias=b2_c, scale=1.0)
                ps3 = tps.tile([ROWS, C], fp32, tag="tr")
                nc.tensor.transpose(ps3, mlpT, identity)
                x3 = tsb.tile([ROWS, C], fp32, tag="x3")
                nc.vector.tensor_add(x3, x2n, ps3)
                xout_t = tsb.tile([ROWS, C], fp32, tag="xo")
                ln_tile(tsb, xout_t, x3, g5_b, b5_b)
                nc.sync.dma_start(xout_o, xout_t)

    nc.compile()
    return nc


_NC_CACHE = {}


def _get_nc(repeat=1):
    if repeat not in _NC_CACHE:
        _NC_CACHE[repeat] = _build_bass(repeat)
    return _NC_CACHE[repeat]


def _shard_inputs(inputs):
    """Build per-core in_maps from full inputs."""
    x = np.asarray(inputs["x"], np.float32)
    y = np.asarray(inputs["y"], np.float32)
    weights = {
        k: np.asarray(inputs[k], np.float32)
        for k in (
            "Wq", "bq", "Wk", "bk", "Wv", "bv", "We", "be", "Won", "bon",
            "Woe", "boe", "W1", "b1", "W2", "b2",
            "ln1_g", "ln1_b", "ln3_g", "ln3_b", "ln5_g", "ln5_b",
        )
    }
    in_maps = []
    for core in range(NCORES):
        g, hhalf = core // 2, core % 2
        r0 = hhalf * ROWS
        m = {
            # (nq, c, nk): channel axis onto partitions, contiguous key axis
            "y_t": np.ascontiguousarray(
                y[g, r0 : r0 + ROWS].transpose(0, 2, 1)
            ),
            "x_g": np.ascontiguousarray(x[g]),
            "x_q": np.ascontiguousarray(x[g, r0 : r0 + ROWS]),
        }
        m.update(weights)
        in_maps.append(m)
    return in_maps


def benchmark(inputs, iters=8, repeat=1):
    """Best wall time of one sharded dispatch of the repeat-variant NEFF
    (includes ~fixed host/tunnel overhead; difference two repeat values to
    isolate on-device execution time)."""
    import time

    import jax
    import numpy as np
    from jax.sharding import Mesh, PartitionSpec
    from jax.experimental.shard_map import shard_map

    import concourse.mybir as mybir
    from concourse.bass2jax import (
        _bass_exec_p,
        install_neuronx_cc_hook,
        partition_id_tensor,
    )

    install_neuronx_cc_hook()
    nc = _get_nc(repeat)
    in_maps = _shard_inputs(inputs)
    partition_name = nc.partition_id_tensor.name if nc.partition_id_tensor else None

    in_names, out_names, out_avals, zero_outs = [], [], [], []
    for alloc in nc.m.functions[0].allocations:
        if not isinstance(alloc, mybir.MemoryLocationSet):
            continue
        name = alloc.memorylocations[0].name
        if alloc.kind == "ExternalInput":
            if name != partition_name:
                in_names.append(name)
        elif alloc.kind == "ExternalOutput":
            out_names.append(name)
            shape = tuple(alloc.tensor_shape)
            dtype = mybir.dt.np(alloc.dtype)
            out_avals.append(jax.core.ShapedArray(shape, dtype))
            zero_outs.append(np.zeros(shape, dtype))
    n_params = len(in_names)
    all_in_names = in_names + out_names
    if partition_name is not None:
        all_in_names = all_in_names + [partition_name]

    def _body(*args):
        operands = list(args)
        if partition_name is not None:
            operands.append(partition_id_tensor())
        outs = _bass_exec_p.bind(
            *operands,
            out_avals=tuple(out_avals),
            in_names=tuple(all_in_names),
            out_names=tuple(out_names),
            lowering_input_output_aliases=(),
            sim_require_finite=True,
            sim_require_nnan=True,
            nc=nc,
        )
        return tuple(outs)

    devices = jax.devices()[:NCORES]
    mesh = Mesh(np.asarray(devices), ("core",))
    spec = PartitionSpec("core")
    n_all = n_params + len(out_names)

    concat_in = [
        np.concatenate([np.asarray(in_maps[c][nm]) for c in range(NCORES)], axis=0)
        for nm in in_names
    ]
    concat_zero = [
        np.zeros((NCORES * z.shape[0], *z.shape[1:]), z.dtype) for z in zero_outs
    ]
    sharding = jax.sharding.NamedSharding(mesh, spec)
    dev_args = [jax.device_put(a, sharding) for a in concat_in + concat_zero]

    fn = jax.jit(
        shard_map(
            _body, mesh=mesh, in_specs=(spec,) * n_all,
            out_specs=(spec,) * len(out_names), check_rep=False,
        )
    )
    fn(*dev_args)[0].block_until_ready()  # compile+warm
    all_times = []
    for _ in range(iters):
        t0 = time.perf_counter()
        fn(*dev_args)[0].block_until_ready()
        all_times.append((time.perf_counter() - t0) * 1e9)
    if os.environ.get("KERNEL_BENCH_ALL", "0") == "1":
        return all_times
    return min(all_times)





def kernel(**inputs):
    global LAST_RESULTS
    from concourse import bass_utils

    nc = _get_nc()
    in_maps = _shard_inputs(inputs)
    trace = os.environ.get("KERNEL_TRACE", "0") == "1"
    res = bass_utils.run_bass_kernel_spmd(
        nc, in_maps, core_ids=list(range(NCORES)), trace=trace
    )
    LAST_RESULTS = res

    x_out = np.empty((B, N, C), np.float32)
    edges = np.empty((B, N, N, C), np.float32)
    for core in range(NCORES):
        g, hhalf = core // 2, core % 2
        r0 = hhalf * ROWS
        x_out[g, r0 : r0 + ROWS] = res.results[core]["xout_o"]
        edges[g, r0 : r0 + ROWS] = res.results[core]["edges_o"]
    return (x_out, edges)


# revision 34
# speedup vs baseline: 133.6405x; 1.0528x over previous
"""Trainium2 Bass kernel for nn_Encoder_Block (gnn_message_passing).

Graph-transformer encoder block:
  x1 = LN1(x); q,k,v = x1@W*; e = y@We
  attn = (q*k*scale) * (e+1)*e          (elementwise, per (nq,nk,c))
  edges = attn@Woe + boe
  p = softmax(attn, axis=nk); agg = sum_k p*v
  x2 = LN3(x1 + agg@Won + bon); x_out = LN5(x2 + MLP(x2))

Sharding: 8 cores; core i handles graph g=i//2, query rows [128*(i%2), 128*(i%2)+128).
Each core is fully independent (data parallel); host gathers.

Per-core layout: channels-on-partitions [c=128, nk] so that
  - softmax reduces along the free axis,
  - q-scaling is a per-partition scalar,
  - d4 = 4*(e^2+e) (.) kT serves directly as matmul lhsT for natural-layout edges.

Key identities (per query node nq):
  sq  = (2*(E+be) + 1)^2 = 4*G + 1 where G = e^2 + e     (one ACT op from PSUM)
  d4  = (sq - 1) (.) kT = 4*G (.) kT                     (one DVE scalar_tensor_tensor)
  attn[c,k] = q4[c] * d4[c,k],  q4 = (q+bq)*scale/4
  p = Exp(d4 * q4_scale), s = sum_k p                    (one ACT op w/ accum)
  a = sum_k p (.) vT                                     (one DVE tensor_tensor_reduce)
  edges[k,j] = sum_c d4[c,k] * (q4[c]*Woe[c,j]) + boe[j] (PE matmuls, natural layout out)
"""

import os

import numpy as np

# ---- problem constants (hardcoded per contract) ----
B, N, C, H = 4, 256, 128, 8
DK = C // H
SCALE = 1.0 / float(np.sqrt(DK))  # 0.25
S4 = SCALE / 4.0
HID = 4 * C
EPS = 1e-5
NCORES = 8
ROWS = N // 2  # 128 query rows per core
NBLK = 4       # query nodes per inner block

LAST_RESULTS = None  # test harness introspection


def _build_bass(repeat=1):
    hwdge = os.environ.get("KERNEL_HWDGE", "0") == "1"
    perf_probe = os.environ.get("KERNEL_PERF_PROBE", "")
    import concourse.bacc as bacc
    import concourse.bass as bass
    import concourse.mybir as mybir
    import concourse.tile as tile
    from concourse.bass import ts
    from concourse.masks import make_identity

    fp32 = mybir.dt.float32
    bf16 = mybir.dt.bfloat16
    A = mybir.AluOpType
    F = mybir.ActivationFunctionType

    nc = bacc.Bacc("TRN2", target_bir_lowering=False, debug=False)

    def din(name, shape):
        return nc.dram_tensor(name, shape, fp32, kind="ExternalInput").ap()

    # y shard pre-transposed on host to (nq, c, nk) so the channel axis lands
    # on partitions with a contiguous cast-DMA (no on-device transpose).
    y_t = din("y_t", (ROWS, C, N))
    x_g = din("x_g", (N, C))
    x_q = din("x_q", (ROWS, C))
    Wq = din("Wq", (C, C)); bq = din("bq", (C,))
    Wk = din("Wk", (C, C)); bk = din("bk", (C,))
    Wv = din("Wv", (C, C)); bv = din("bv", (C,))
    We = din("We", (C, C)); be = din("be", (C,))
    Won = din("Won", (C, C)); bon = din("bon", (C,))
    Woe = din("Woe", (C, C)); boe = din("boe", (C,))
    W1 = din("W1", (C, HID)); b1 = din("b1", (HID,))
    W2 = din("W2", (HID, C)); b2 = din("b2", (C,))
    ln1_g = din("ln1_g", (C,)); ln1_b = din("ln1_b", (C,))
    ln3_g = din("ln3_g", (C,)); ln3_b = din("ln3_b", (C,))
    ln5_g = din("ln5_g", (C,)); ln5_b = din("ln5_b", (C,))

    edges_o = nc.dram_tensor("edges_o", (ROWS, N, C), fp32, kind="ExternalOutput").ap()
    xout_o = nc.dram_tensor("xout_o", (ROWS, C), fp32, kind="ExternalOutput").ap()

    def bcast_ap(vec_ap, p=128):
        # [C] dram vector broadcast across p partitions -> [p, C]
        return bass.AP(
            tensor=vec_ap.tensor, offset=vec_ap.offset, ap=[[0, p]] + list(vec_ap.ap)
        )

    def col_ap(vec_ap):
        # [C] dram vector as a [C, 1] partition-column
        return vec_ap.unsqueeze(1)

    with tile.TileContext(nc) as tc:
        with tc.tile_pool(name="singles", bufs=1) as singles:
            # ---------- load constants ----------
            We_bf = singles.tile([C, C], bf16)
            nc.gpsimd.dma_start(We_bf, We)
            We_f = singles.tile([C, C], fp32)
            nc.sync.dma_start(We_f, We)
            Woe_bf = singles.tile([C, C], bf16)
            nc.gpsimd.dma_start(Woe_bf, Woe)
            Wq_t = singles.tile([C, C], fp32); nc.sync.dma_start(Wq_t, Wq)
            Wk_t = singles.tile([C, C], fp32); nc.sync.dma_start(Wk_t, Wk)
            Wv_t = singles.tile([C, C], fp32); nc.sync.dma_start(Wv_t, Wv)
            Won_t = singles.tile([C, C], fp32); nc.sync.dma_start(Won_t, Won)
            W1_t = singles.tile([C, HID], fp32); nc.sync.dma_start(W1_t, W1)
            W2_t = singles.tile([C, HID // C, C], fp32)
            nc.sync.dma_start(W2_t, W2.rearrange("(t p) c -> p t c", p=C))

            bq_c = singles.tile([C, 1], fp32); nc.sync.dma_start(bq_c, col_ap(bq))
            bk_c = singles.tile([C, 1], fp32); nc.sync.dma_start(bk_c, col_ap(bk))
            bv_c = singles.tile([C, 1], fp32); nc.sync.dma_start(bv_c, col_ap(bv))
            be_c = singles.tile([C, 1], fp32); nc.sync.dma_start(be_c, col_ap(be))
            bon_c = singles.tile([C, 1], fp32); nc.sync.dma_start(bon_c, col_ap(bon))
            b2_c = singles.tile([C, 1], fp32); nc.sync.dma_start(b2_c, col_ap(b2))
            b1_t = singles.tile([C, HID // C], fp32)
            nc.sync.dma_start(b1_t, b1.rearrange("(t p) -> p t", p=C))

            # boe repeated 4x in partition 0 (rhs of the rank-1 bias matmul); bf16
            boe4 = singles.tile([1, 4 * C], bf16)
            for r in range(4):
                nc.gpsimd.dma_start(boe4[:, r * C : (r + 1) * C], boe.unsqueeze(0))
            ones1 = singles.tile([1, C], bf16)
            nc.vector.memset(ones1, 1.0)

            g1_b = singles.tile([C, C], fp32); nc.sync.dma_start(g1_b, bcast_ap(ln1_g))
            b1_b = singles.tile([C, C], fp32); nc.sync.dma_start(b1_b, bcast_ap(ln1_b))
            g3_b = singles.tile([C, C], fp32); nc.sync.dma_start(g3_b, bcast_ap(ln3_g))
            b3_b = singles.tile([C, C], fp32); nc.sync.dma_start(b3_b, bcast_ap(ln3_b))
            g5_b = singles.tile([C, C], fp32); nc.sync.dma_start(g5_b, bcast_ap(ln5_g))
            b5_b = singles.tile([C, C], fp32); nc.sync.dma_start(b5_b, bcast_ap(ln5_b))

            eps_t = singles.tile([C, 1], fp32)
            nc.vector.memset(eps_t, EPS)

            identity = singles.tile([C, C], fp32)
            make_identity(nc, identity[:])

            # b2e = 2*be + 1  (bias for the Square trick)
            b2e = singles.tile([C, 1], fp32)
            nc.vector.tensor_scalar(b2e, be_c, 2.0, 1.0, A.mult, A.add)
            # bq4 = bq * scale/4
            bq4 = singles.tile([C, 1], fp32)
            nc.vector.tensor_scalar_mul(bq4, bq_c, S4)

            # persistent activations
            kT_bf = singles.tile([C, N], bf16)
            vT_bf = singles.tile([C, N], bf16)
            q4 = singles.tile([C, ROWS], fp32)
            aggT = singles.tile([C, ROWS], fp32)
            if perf_probe == "skip_softmax":
                nc.vector.memset(aggT, 0.0)
            x1q = singles.tile([ROWS, C], fp32)

            def ln_tile(pool, out, x_tile, g_bc, b_bc):
                st = pool.tile([C, 6], fp32, tag="ln_stats")
                nc.vector.bn_stats(st, x_tile)
                mv = pool.tile([C, 2], fp32, tag="ln_mv")
                nc.vector.bn_aggr(mv, st)
                sd = pool.tile([C, 1], fp32, tag="ln_sd")
                nc.scalar.activation(sd, mv[:, 1:2], F.Sqrt, bias=eps_t, scale=1.0)
                rstd = pool.tile([C, 1], fp32, tag="ln_rstd")
                nc.vector.reciprocal(rstd, sd)
                nc.vector.tensor_scalar(
                    out, x_tile, mv[:, 0:1], rstd, A.subtract, A.mult
                )
                nc.vector.tensor_mul(out, out, g_bc)
                nc.vector.tensor_add(out, out, b_bc)

            # ---------- setup: x-path projections ----------
            with (
                tc.tile_pool(name="setup_sb", bufs=2) as ssb,
                tc.tile_pool(name="setup_ps", bufs=2, space="PSUM") as sps,
            ):
                # LN1 over the full graph (for k, v) and over this core's rows (q)
                x1T = singles.tile([C, N], fp32)
                for t in range(2):
                    xg_t = ssb.tile([C, C], fp32, tag="xg")
                    nc.sync.dma_start(xg_t, x_g[ts(t, C)])
                    x1g = ssb.tile([C, C], fp32, tag="x1g")
                    ln_tile(ssb, x1g, xg_t, g1_b, b1_b)
                    ps = sps.tile([C, C], fp32, tag="tps")
                    nc.tensor.transpose(ps, x1g, identity)
                    nc.scalar.copy(x1T[:, ts(t, C)], ps)

                xq_t = ssb.tile([C, C], fp32, tag="xq")
                nc.sync.dma_start(xq_t, x_q)
                ln_tile(ssb, x1q, xq_t, g1_b, b1_b)
                x1qT = singles.tile([C, ROWS], fp32)
                ps = sps.tile([C, C], fp32, tag="tps")
                nc.tensor.transpose(ps, x1q, identity)
                nc.scalar.copy(x1qT, ps)

                # kT/vT (full graph), q4 (this core's rows)
                kv_ps = sps.tile([C, N], fp32, tag="kv")
                nc.tensor.matmul(kv_ps, Wk_t, x1T, start=True, stop=True)
                nc.scalar.activation(kT_bf, kv_ps, F.Identity, bias=bk_c, scale=1.0)
                kv_ps2 = sps.tile([C, N], fp32, tag="kv")
                nc.tensor.matmul(kv_ps2, Wv_t, x1T, start=True, stop=True)
                nc.scalar.activation(vT_bf, kv_ps2, F.Identity, bias=bv_c, scale=1.0)
                q_ps = sps.tile([C, ROWS], fp32, tag="q")
                nc.tensor.matmul(q_ps, Wq_t, x1qT, start=True, stop=True)
                nc.scalar.activation(q4, q_ps, F.Identity, bias=bq4, scale=S4)

            # ---------- main loop over query nodes ----------
            nbuf = int(os.environ.get("KERNEL_BUFS", "3"))
            with (
                tc.tile_pool(name="yt", bufs=nbuf) as yt_pool,
                tc.tile_pool(name="sqp", bufs=nbuf) as sq_pool,
                tc.tile_pool(name="d4p", bufs=nbuf * NBLK) as d4_pool,
                tc.tile_pool(name="dqwp", bufs=nbuf * NBLK) as dqw_pool,
                tc.tile_pool(name="pp", bufs=nbuf * NBLK) as p_pool,
                tc.tile_pool(name="pvp", bufs=3) as pv_pool,
                tc.tile_pool(name="edsb", bufs=nbuf) as edsb_pool,
                tc.tile_pool(name="stat", bufs=4) as stat_pool,
                tc.tile_pool(name="psE", bufs=2, space="PSUM") as psum_e,
                tc.tile_pool(name="psED", bufs=2, space="PSUM") as psum_ed,
            ):
                for blk in range(repeat * (ROWS // NBLK)):
                    nq0 = (blk % (ROWS // NBLK)) * NBLK
                    yT = yt_pool.tile([C, NBLK, N], fp32 if hwdge else bf16)
                    dma_in = nc.sync.dma_start if hwdge else nc.gpsimd.dma_start
                    dma_in(yT, y_t[nq0 : nq0 + NBLK].rearrange("b c k -> c b k"))

                    e_ps = psum_e.tile([C, NBLK, N], fp32)
                    for half in range(NBLK // 2):
                        nc.tensor.matmul(
                            e_ps[:, 2 * half : 2 * half + 2],
                            We_f if hwdge else We_bf,
                            yT[:, 2 * half : 2 * half + 2],
                            start=True,
                            stop=True,
                        )
                    sq = sq_pool.tile([C, NBLK, N], bf16)
                    nc.scalar.activation(sq, e_ps, F.Square, bias=b2e, scale=2.0)

                    ed_ps = psum_ed.tile([C, NBLK, 2, C], fp32)
                    s_blk = stat_pool.tile([C, NBLK], fp32, tag="s")
                    a_blk = stat_pool.tile([C, NBLK], fp32, tag="a")
                    # stage-major issue order: keeps each engine's stream free of
                    # cross-engine waits mid-block (per-engine streams execute in
                    # issue order, so nq-major order stalls DVE on ACT every node)
                    d4s, p_ts = [], []
                    for bi in range(NBLK):
                        nq = nq0 + bi
                        d4 = d4_pool.tile([C, N], bf16)
                        nc.vector.scalar_tensor_tensor(
                            d4, sq[:, bi], -1.0, kT_bf, A.add, A.mult
                        )
                        d4s.append(d4)
                        dqw = dqw_pool.tile([C, C], bf16)
                        nc.vector.tensor_scalar_mul(dqw, Woe_bf, q4[:, nq : nq + 1])
                        if bi % 2 == 0:
                            # bias prefill: one rank-1 matmul per pair of nodes
                            nc.tensor.matmul(
                                ed_ps[:, bi : bi + 2],
                                ones1,
                                boe4,
                                start=True,
                                stop=False,
                            )
                        for t in range(2):
                            nc.tensor.matmul(
                                ed_ps[:, bi, t],
                                d4[:, ts(t, C)],
                                dqw,
                                start=False,
                                stop=(bi % 2 == 1 and t == 1),
                            )
                    if perf_probe != "skip_softmax":
                        for bi in range(NBLK):
                            nq = nq0 + bi
                            p_t = p_pool.tile([C, N], bf16)
                            nc.scalar.activation(
                                p_t,
                                d4s[bi],
                                F.Exp,
                                scale=q4[:, nq : nq + 1],
                                accum_out=s_blk[:, bi : bi + 1],
                            )
                            p_ts.append(p_t)
                        for bi in range(NBLK):
                            pv = pv_pool.tile([C, N], bf16)
                            nc.vector.scalar_tensor_tensor(
                                pv,
                                p_ts[bi],
                                1.0,
                                vT_bf,
                                A.mult,
                                A.mult,
                                accum_out=a_blk[:, bi : bi + 1],
                            )

                    ed_sb = edsb_pool.tile([C, NBLK, 2, C], fp32 if hwdge else bf16)
                    copy_mode = os.environ.get("KERNEL_COPY_ENGINE", "alternate")
                    use_act = copy_mode == "act" or (
                        copy_mode == "alternate" and blk % 2 == 0
                    )
                    if use_act:
                        nc.scalar.copy(ed_sb, ed_ps)
                    else:
                        nc.vector.tensor_copy(ed_sb, ed_ps)
                    dma_out = nc.sync.dma_start if hwdge else nc.gpsimd.dma_start
                    dma_out(
                        edges_o[nq0 : nq0 + NBLK].rearrange(
                            "b (t p) c -> p b t c", p=C
                        ),
                        ed_sb,
                    )

                    if perf_probe != "skip_softmax":
                        rec = stat_pool.tile([C, NBLK], fp32, tag="rec")
                        nc.vector.reciprocal(rec, s_blk)
                        nc.vector.tensor_mul(aggT[:, nq0 : nq0 + NBLK], a_blk, rec)

            # ---------- tail: node update + MLP ----------
            with (
                tc.tile_pool(name="tail_sb", bufs=1) as tsb,
                tc.tile_pool(name="tail_ps", bufs=1, space="PSUM") as tps,
            ):
                nodes_ps = tps.tile([C, ROWS], fp32, tag="m")
                nc.tensor.matmul(nodes_ps, Won_t, aggT, start=True, stop=True)
                nodesT = tsb.tile([C, ROWS], fp32, tag="t0")
                nc.scalar.activation(nodesT, nodes_ps, F.Identity, bias=bon_c, scale=1.0)
                ps = tps.tile([ROWS, C], fp32, tag="tr")
                nc.tensor.transpose(ps, nodesT, identity)
                x2 = tsb.tile([ROWS, C], fp32, tag="x2")
                nc.vector.tensor_add(x2, x1q, ps)
                x2n = tsb.tile([ROWS, C], fp32, tag="x2n")
                ln_tile(tsb, x2n, x2, g3_b, b3_b)
                ps2 = tps.tile([ROWS, C], fp32, tag="tr")
                nc.tensor.transpose(ps2, x2n, identity)
                x2nT = tsb.tile([C, ROWS], fp32, tag="t1")
                nc.scalar.copy(x2nT, ps2)

                h_ps = tps.tile([C, HID // C, ROWS], fp32, tag="h")
                hr = tsb.tile([C, HID // C, ROWS], fp32, tag="hr")
                for m in range(HID // C):
                    nc.tensor.matmul(
                        h_ps[:, m], W1_t[:, ts(m, C)], x2nT, start=True, stop=True
                    )
                    nc.scalar.activation(
                        hr[:, m], h_ps[:, m], F.Relu, bias=b1_t[:, m : m + 1], scale=1.0
                    )
                mlp_ps = tps.tile([C, ROWS], fp32, tag="m")
                for t in range(HID // C):
                    nc.tensor.matmul(
                        mlp_ps,
                        W2_t[:, t],
                        hr[:, t],
                        start=(t == 0),
                        stop=(t == HID // C - 1),
                    )
                mlpT = tsb.tile([C, ROWS], fp32, tag="t0")
                nc.scalar.activation(mlpT, mlp_ps, F.Identity, bias=b2_c, scale=1.0)
                ps3 = tps.tile([ROWS, C], fp32, tag="tr")
                nc.tensor.transpose(ps3, mlpT, identity)
                x3 = tsb.tile([ROWS, C], fp32, tag="x3")
                nc.vector.tensor_add(x3, x2n, ps3)
                xout_t = tsb.tile([ROWS, C], fp32, tag="xo")
                ln_tile(tsb, xout_t, x3, g5_b, b5_b)
                nc.sync.dma_start(xout_o, xout_t)

    nc.compile()
    return nc


_NC_CACHE = {}


def _get_nc(repeat=1):
    if repeat not in _NC_CACHE:
        _NC_CACHE[repeat] = _build_bass(repeat)
    return _NC_CACHE[repeat]


def _shard_inputs(inputs):
    """Build per-core in_maps from full inputs."""
    x = np.asarray(inputs["x"], np.float32)
    y = np.asarray(inputs["y"], np.float32)
    weights = {
        k: np.asarray(inputs[k], np.float32)
        for k in (
            "Wq", "bq", "Wk", "bk", "Wv", "bv", "We", "be", "Won", "bon",
            "Woe", "boe", "W1", "b1", "W2", "b2",
            "ln1_g", "ln1_b", "ln3_g", "ln3_b", "ln5_g", "ln5_b",
        )
    }
    in_maps = []
    for core in range(NCORES):
        g, hhalf = core // 2, core % 2
        r0 = hhalf * ROWS
        m = {
            # (nq, c, nk): channel axis onto partitions, contiguous key axis
            "y_t": np.ascontiguousarray(
                y[g, r0 : r0 + ROWS].transpose(0, 2, 1)
            ),
            "x_g": np.ascontiguousarray(x[g]),
            "x_q": np.ascontiguousarray(x[g, r0 : r0 + ROWS]),
        }
        m.update(weights)
        in_maps.append(m)
    return in_maps


def benchmark(inputs, iters=8, repeat=1):
    """Best wall time of one sharded dispatch of the repeat-variant NEFF
    (includes ~fixed host/tunnel overhead; difference two repeat values to
    isolate on-device execution time)."""
    import time

    import jax
    import numpy as np
    from jax.sharding import Mesh, PartitionSpec
    from jax.experimental.shard_map import shard_map

    import concourse.mybir as mybir
    from concourse.bass2jax import (
        _bass_exec_p,
        install_neuronx_cc_hook,
        partition_id_tensor,
    )

    install_neuronx_cc_hook()
    nc = _get_nc(repeat)
    in_maps = _shard_inputs(inputs)
    partition_name = nc.partition_id_tensor.name if nc.partition_id_tensor else None

    in_names, out_names, out_avals, zero_outs = [], [], [], []
    for alloc in nc.m.functions[0].allocations:
        if not isinstance(alloc, mybir.MemoryLocationSet):
            continue
        name = alloc.memorylocations[0].name
        if alloc.kind == "ExternalInput":
            if name != partition_name:
                in_names.append(name)
        elif alloc.kind == "ExternalOutput":
            out_names.append(name)
            shape = tuple(alloc.tensor_shape)
            dtype = mybir.dt.np(alloc.dtype)
            out_avals.append(jax.core.ShapedArray(shape, dtype))
            zero_outs.append(np.zeros(shape, dtype))
    n_params = len(in_names)
    all_in_names = in_names + out_names
    if partition_name is not None:
        all_in_names = all_in_names + [partition_name]

    def _body(*args):
        operands = list(args)
        if partition_name is not None:
            operands.append(partition_id_tensor())
        outs = _bass_exec_p.bind(
            *operands,
            out_avals=tuple(out_avals),
            in_names=tuple(all_in_names),
            out_names=tuple(out_names),
            lowering_input_output_aliases=(),
            sim_require_finite=True,
            sim_require_nnan=True,
            nc=nc,
        )
        return tuple(outs)

    devices = jax.devices()[:NCORES]
    mesh = Mesh(np.asarray(devices), ("core",))
    spec = PartitionSpec("core")
    n_all = n_params + len(out_names)

    concat_in = [
        np.concatenate([np.asarray(in_maps[c][nm]) for c in range(NCORES)], axis=0)
        for nm in in_names
    ]
    concat_zero = [
        np.zeros((NCORES * z.shape[0], *z.shape[1:]), z.dtype) for z in zero_outs
    ]
    sharding = jax.sharding.NamedSharding(mesh, spec)
    dev_args = [jax.device_put(a, sharding) for a in concat_in + concat_zero]

    fn = jax.jit(
        shard_map(
            _body, mesh=mesh, in_specs=(spec,) * n_all,
            out_specs=(spec,) * len(out_names), check_rep=False,
        )
    )
    fn(*dev_args)[0].block_until_ready()  # compile+warm
    all_times = []
    for _ in range(iters):
        t0 = time.perf_counter()
        fn(*dev_args)[0].block_until_ready()
        all_times.append((time.perf_counter() - t0) * 1e9)
    if os.environ.get("KERNEL_BENCH_ALL", "0") == "1":
        return all_times
    return min(all_times)





def kernel(**inputs):
    global LAST_RESULTS
    from concourse import bass_utils

    nc = _get_nc()
    in_maps = _shard_inputs(inputs)
    trace = os.environ.get("KERNEL_TRACE", "0") == "1"
    res = bass_utils.run_bass_kernel_spmd(
        nc, in_maps, core_ids=list(range(NCORES)), trace=trace
    )
    LAST_RESULTS = res

    x_out = np.empty((B, N, C), np.float32)
    edges = np.empty((B, N, N, C), np.float32)
    for core in range(NCORES):
        g, hhalf = core // 2, core % 2
        r0 = hhalf * ROWS
        x_out[g, r0 : r0 + ROWS] = res.results[core]["xout_o"]
        edges[g, r0 : r0 + ROWS] = res.results[core]["edges_o"]
    return (x_out, edges)
